# revision 11
# baseline (speedup 1.0000x reference)
"""Photonic-mesh (NEUROPULS) chain kernel for Trainium2, 8 NeuronCores.

Sequential chain of 128 fused stages; each stage applies a per-pair 2x2
complex mix (heaters+MMIs) then a crossing that couples adjacent pairs.
Columns of the accumulated arch matrix propagate independently: 16
columns per core, split into two 8-column chains that interleave so
engines overlap.

Per stage per chain, with gauge state (E~ = E, O~ = wt*O, wt^2 = 99):
  DVE (4 custom ops, fp16 out):  te1=A(.)E~ te2=B(.)O~ to1=G(.)E~ to2=D(.)O~
  PE  (fp16, PSUM acc, exact weights I/+-S_dn/+-99*S_up):
      psE = te1+te2 + i*S_dn(to1+to2)   [i via +- half-column matmuls]
      psO = to1+to2 + i*99*S_up(te1+te2) * (1/wt gauge)
  ACT: copy psE->E~', psO->O~'  (fp32 SBUF)
All per-stage physics constants live in the per-partition fp32 DVE
scalars (exact); PE weights are exactly representable in fp16.
"""

import math

import numpy as np

import concourse.bass as bass
import concourse.mybir as mybir
from concourse.ap import AP

N = 128
NCORES = 8
COLS = 16            # columns per core
H = 8                # columns per chain (2 chains per core)
NST = 128            # C-stages (0..127); crossings after stages 0..126
NSTAGES = 129        # + projection stage

IL_MMI = 0.05
IMB = 0.005
IL_X = 0.02
CT = 0.01
WT = math.sqrt((1.0 - CT) / CT)   # wt^2 = 99 exactly

F32 = mybir.dt.float32
F16 = mybir.dt.float16

# ----------------------------------------------------------------------------
# custom DVE op: out[p,s,k] = in1[p, s*H+k]*s0[p] + in0[p,s,k]*s1[p]*(2s-1)
# with in1 = natural [re|im] view and in0 = page-swapped view this is a full
# per-partition complex scale (s0 + i*s1) (.) x in one op.
# ----------------------------------------------------------------------------
_CMULA = None


def _get_cmula():
    global _CMULA
    if _CMULA is not None:
        return _CMULA
    import concourse.dve_ops as dom
    from concourse.dve_ops import OPS, DveOp
    from concourse.dve_spec import Spec, Src0, Src1, C0, C1, SubIdx, One, lower
    from concourse.dve_uop import DveOpSpec

    name = "CMULA_NP_ANT"
    for op in OPS:  # idempotent across re-imports
        if op.name == name:
            _CMULA = op
            return op

    def _ref(in0, in1, s0, s1, imm2):
        pg = (np.arange(in0.shape[1], dtype=np.float32) * 2.0 - 1.0).reshape(1, -1, 1)
        a = np.asarray(s0, np.float32).reshape(-1, 1, 1) if np.ndim(s0) else np.float32(s0)
        b = np.asarray(s1, np.float32).reshape(-1, 1, 1) if np.ndim(s1) else np.float32(s1)
        return (np.asarray(in1, np.float32).reshape(in0.shape) * a
                + np.asarray(in0, np.float32) * b * pg).astype(np.float32)

    op = DveOp(
        name,
        Spec(body=Src1 * C0 + Src0 * C1 * (SubIdx + SubIdx - One), reference=_ref),
        subdim=True,
        uops_sha={},
    )
    OPS.append(op)
    dom._SUB_OPCODE_FOR_NAME[name] = dom._CUSTOM_DVE_ROW_BASE + len(OPS) - 1
    dom.CUSTOM_DVE_SPECS[name] = op.spec
    for ver in ("v3", "v4"):
        spec_c = DveOpSpec(name=name, opcode=dom.get_dve_sub_opcode(name),
                           uops=lower(op.spec, ver=ver), rd1_en=True)
        op.uops_sha[ver] = spec_c.sha(ver)
    _CMULA = op
    return op


def _nat3(t, h=H):
    """[P, 2, h] natural-page view of a [P, 2h] AP."""
    return AP(t.tensor, t.offset, [list(t.ap[0]), [h, 2], [1, h]])


def _swp3(t, h=H):
    """[P, 2, h] page-swapped view of a [P, 2h] AP (page0 = imag half)."""
    return AP(t.tensor, t.offset + h, [list(t.ap[0]), [-h, 2], [1, h]])


# ----------------------------------------------------------------------------
# device program (input-independent; built once)
# ----------------------------------------------------------------------------
_PROG = None


def _build_program():
    global _PROG
    if _PROG is not None:
        return _PROG
    CMULA = _get_cmula()

    import concourse.bacc as bacc
    nc = bacc.Bacc(None, target_bir_lowering=False)
    d_xe = [nc.declare_dram_parameter(f"xe{c}", [N, 2 * H], F32, isOutput=False)
            for c in range(2)]
    d_xo = [nc.declare_dram_parameter(f"xo{c}", [N, 2 * H], F32, isOutput=False)
            for c in range(2)]
    d_coef = [nc.declare_dram_parameter(f"coef{i}", [N, NSTAGES], F32, isOutput=False)
              for i in range(8)]
    d_drr = nc.declare_dram_parameter("drr", [N, 2 * NST], F32, isOutput=False)
    d_dsg = nc.declare_dram_parameter("dsg", [N, 2 * NST], F32, isOutput=False)
    d_w = nc.declare_dram_parameter("wts", [N, 5 * N], F16, isOutput=False)
    d_out = [nc.declare_dram_parameter(f"out{c}", [N, 2 * H], F32, isOutput=True)
             for c in range(2)]

    from concourse import tile

    with tile.TileContext(nc) as tc:
        with (tc.tile_pool(name="const", bufs=1) as cpool,
              tc.tile_pool(name="state", bufs=2) as spool,
              tc.tile_pool(name="tmp", bufs=3) as tpool,
              tc.tile_pool(name="ps", bufs=2, space="PSUM") as ppool):
            coefT = cpool.tile([N, 8 * NSTAGES], F32, tag="coef")
            drr = cpool.tile([N, 2 * NST], F32, tag="drr")
            dsg = cpool.tile([N, 2 * NST], F32, tag="dsg")
            wts = cpool.tile([N, 5 * N], F16, tag="wts")
            for i in range(8):
                nc.sync.dma_start(coefT[:, i * NSTAGES:(i + 1) * NSTAGES],
                                  d_coef[i][:])
            nc.sync.dma_start(drr[:], d_drr[:])
            nc.sync.dma_start(dsg[:], d_dsg[:])
            nc.sync.dma_start(wts[:], d_w[:])
            coef = [coefT[:, i * NSTAGES:(i + 1) * NSTAGES] for i in range(8)]
            wI = wts[:, 0 * N:1 * N]
            wDnM = wts[:, 1 * N:2 * N]   # -S_dn
            wDnP = wts[:, 2 * N:3 * N]   # +S_dn
            wUpM = wts[:, 3 * N:4 * N]   # -99*S_up
            wUpP = wts[:, 4 * N:5 * N]   # +99*S_up

            xeo, outT = [], []
            for c in range(2):
                t1 = spool.tile([N, 4 * H], F32, name=f"xeo_{c}", tag=f"xeo{c}")
                t3 = cpool.tile([N, 2 * H], F32, name=f"outT_{c}", tag=f"outT{c}")
                nc.sync.dma_start(t1[:, 0:2 * H], d_xe[c][:])
                nc.sync.dma_start(t1[:, 2 * H:4 * H], d_xo[c][:])
                xeo.append(t1)
                outT.append(t3)

            def cmul(dst, src, cr, ci):
                return nc.vector._custom_dve(CMULA, out=_nat3(dst[:]),
                                             in0=_swp3(src), in1=src,
                                             s0=cr, s1=ci)

            def bview(t, k):
                b = t[:]
                return AP(b.tensor, b.offset + 2 * k, [list(b.ap[0]), [1, 2], [0, H]])

            for k in range(NST):  # C-stages 0..127
                cc = [coef[i][:, k:k + 1] for i in range(8)]
                last = k == NST - 1
                for c in range(2):
                    xe_v = xeo[c][:, 0:2 * H]
                    xo_v = xeo[c][:, 2 * H:4 * H]
                    m1 = tpool.tile([N, 2 * H], F16, name=f"m1_{c}", tag=f"m1{c}")
                    m2 = tpool.tile([N, 2 * H], F16, name=f"m2_{c}", tag=f"m2{c}")
                    # to2 = D (.) O~ split as Dr*O~ (ACT) + pageswap-sign*Di*O~
                    # (GPSIMD); PE does the add via PSUM accumulation
                    nc.scalar.mul(m1[:], xo_v, cc[6])
                    nc.gpsimd.tensor_tensor(_nat3(m2[:]), _swp3(xo_v), bview(dsg, k),
                                            mybir.AluOpType.mult)
                    te1 = tpool.tile([N, 2 * H], F16, name=f"te1_{c}", tag=f"te1{c}")
                    te2 = tpool.tile([N, 2 * H], F16, name=f"te2_{c}", tag=f"te2{c}")
                    to1 = tpool.tile([N, 2 * H], F16, name=f"to1_{c}", tag=f"to1{c}")
                    cmul(te1, xe_v, cc[0], cc[1])   # A (.) E~
                    cmul(te2, xo_v, cc[2], cc[3])   # B (.) O~
                    cmul(to1, xe_v, cc[4], cc[5])   # G (.) E~
                    ps = ppool.tile([N, 4 * H], F32, name=f"ps_{c}", tag=f"ps{c}")
                    psE = ps[:, 0:2 * H]
                    psO = ps[:, 2 * H:4 * H]
                    # psE = te1+te2 (+ crossing i*S_dn(to1+m1+m2))
                    nc.tensor.matmul(psE, wI, te1[:], start=True, stop=False)
                    nc.tensor.matmul(psE, wI, te2[:], start=False, stop=last)
                    if not last:
                        for t in (to1, m1, m2):
                            nc.tensor.matmul(ps[:, 0:H], wDnM, t[:, H:2 * H],
                                             start=False, stop=False)
                        for i, t in enumerate((to1, m1, m2)):
                            nc.tensor.matmul(ps[:, H:2 * H], wDnP, t[:, 0:H],
                                             start=False, stop=(i == 2))
                    # psO = to1+m1+m2 (+ crossing i*99*S_up(te1+te2))
                    nc.tensor.matmul(psO, wI, to1[:], start=True, stop=False)
                    nc.tensor.matmul(psO, wI, m1[:], start=False, stop=False)
                    nc.tensor.matmul(psO, wI, m2[:], start=False, stop=last)
                    if not last:
                        for t in (te1, te2):
                            nc.tensor.matmul(ps[:, 2 * H:3 * H], wUpM, t[:, H:2 * H],
                                             start=False, stop=False)
                        for i, t in enumerate((te1, te2)):
                            nc.tensor.matmul(ps[:, 3 * H:4 * H], wUpP, t[:, 0:H],
                                             start=False, stop=(i == 1))
                    xeo_n = spool.tile([N, 4 * H], F32, name=f"xeon_{c}", tag=f"xeo{c}")
                    nc.scalar.copy(xeo_n[:], ps[:])
                    xeo[c] = xeo_n

            # projection stage
            cc = [coef[i][:, NST:NST + 1] for i in range(4)]
            for c in range(2):
                tp1 = tpool.tile([N, 2 * H], F16, name=f"tp1_{c}", tag=f"te1{c}")
                tp2 = tpool.tile([N, 2 * H], F16, name=f"tp2_{c}", tag=f"te2{c}")
                cmul(tp1, xeo[c][:, 0:2 * H], cc[0], cc[1])
                cmul(tp2, xeo[c][:, 2 * H:4 * H], cc[2], cc[3])
                psP = ppool.tile([N, 2 * H], F32, name=f"psP_{c}", tag=f"psP{c}")
                nc.tensor.matmul(psP[:], wI, tp1[:], start=True, stop=False)
                nc.tensor.matmul(psP[:], wI, tp2[:], start=False, stop=True)
                nc.scalar.copy(outT[c][:], psP[:])
                nc.sync.dma_start(d_out[c][:], outT[c][:])

    nc.finalize()
    _PROG = nc
    return _PROG


# ----------------------------------------------------------------------------
# host-side coefficient construction
# ----------------------------------------------------------------------------
def _host_inputs(theta_in, theta_even, theta_out):
    theta_in = np.asarray(theta_in, np.float64)
    theta_even = np.asarray(theta_even, np.float64)
    theta_out = np.asarray(theta_out, np.float64)

    aM = math.sqrt(1.0 - IL_MMI)
    bp = aM * math.sqrt(0.5 + IMB)
    bq = aM * math.sqrt(0.5 - IMB)
    B = np.array([[bp, 1j * bq], [1j * bq, bp]], np.complex128)
    aX = math.sqrt(1.0 - IL_X)
    u = aX * math.sqrt(CT)
    vv = aX * math.sqrt(1.0 - CT)

    ph = np.exp(1j * theta_even)  # [255, 128]

    Cs = np.zeros((NSTAGES, N, 2, 2), np.complex128)
    # stage 0: B @ diag(a0, 1)
    Cs[0, :, :, 0] = B[:, 0][None, :] * ph[0][:, None]
    Cs[0, :, :, 1] = B[:, 1][None, :]
    # stages 1..126: (B @ diag(b,1)) @ (B @ diag(a,1)),  a=ph[2i-1], b=ph[2i]
    i = np.arange(1, N - 1)
    a = ph[2 * i - 1]  # [126, 128]
    b = ph[2 * i]
    T1 = np.zeros((N - 2, N, 2, 2), np.complex128)
    T1[:, :, :, 0] = B[:, 0][None, None, :] * a[:, :, None]
    T1[:, :, :, 1] = B[:, 1][None, None, :]
    T2 = np.zeros_like(T1)
    T2[:, :, :, 0] = B[:, 0][None, None, :] * b[:, :, None]
    T2[:, :, :, 1] = B[:, 1][None, None, :]
    Cs[1:N - 1] = np.einsum("sjab,sjbc->sjac", T2, T1)
    # stage 127: half epilogue B @ diag(ph[253], 1)
    Cs[N - 1, :, :, 0] = B[:, 0][None, :] * ph[2 * N - 3][:, None]
    Cs[N - 1, :, :, 1] = B[:, 1][None, :]
    # stage 128: projection  out = f0*E + f1*O
    f0 = np.exp(1j * theta_out) * bp * ph[2 * N - 2]
    f1 = np.exp(1j * theta_out) * (1j * bq)
    Cs[N, :, 0, 0] = f0
    Cs[N, :, 0, 1] = f1

    # fold crossing scalars/corners of K-stage s (s=0..126) into stage s+1
    dE = np.full(N, u); dE[0] = vv
    dO = np.full(N, u); dO[N - 1] = vv
    Cs[1:N, :, :, 0] *= dE[None, :, None]
    Cs[1:N, :, :, 1] *= dO[None, :, None]

    # gauge + normalization:
    #   state O~ = wt*O;  per-stage amplitude renorm g_k keeps fp16 in range.
    # device coefficients per stage k:
    #   A = g*C00, B = g*C01/wt, G = g*wt*C10, D = g*C11
    # projection: A = C00/Pg, B = C01/(wt*Pg)   (Pg = prod of g_k)
    A = np.zeros((NST + 1, N), np.complex128)
    Bc = np.zeros((NST + 1, N), np.complex128)
    G = np.zeros((NST, N), np.complex128)
    D = np.zeros((NST, N), np.complex128)
    Pg = 1.0
    for k in range(NST):
        g = 1.0 / (aM * aM * aX)  # typical per-stage amplitude loss
        A[k] = g * Cs[k, :, 0, 0]
        Bc[k] = g * Cs[k, :, 0, 1] / WT
        G[k] = g * WT * Cs[k, :, 1, 0]
        D[k] = g * Cs[k, :, 1, 1]
        Pg *= g
    A[NST] = Cs[NST, :, 0, 0] / Pg
    Bc[NST] = Cs[NST, :, 0, 1] / (WT * Pg)

    coefs = [np.ascontiguousarray(x.astype(np.float32)) for x in (
        A.T.real, A.T.imag, Bc.T.real, Bc.T.imag,
        np.concatenate([G, G[:1]]).T.real, np.concatenate([G, G[:1]]).T.imag,
        np.concatenate([D, D[:1]]).T.real, np.concatenate([D, D[:1]]).T.imag,
    )]

    # initial state: columns of MMI_IN @ diag(exp(i theta_in)); O~ = wt*O
    din = np.exp(1j * theta_in)
    E0 = np.zeros((N, N), np.complex128)
    O0 = np.zeros((N, N), np.complex128)
    E0[np.arange(N), np.arange(N)] = bp * din
    O0[np.arange(N), np.arange(N)] = WT * (1j * bq) * din

    # PE weights (all exact in fp16):
    #   psA[j] = x[j+1]  (S_up):  lhsT up[j+1, j] = 1
    #   psB[j] = x[j-1]  (S_dn):  lhsT dn[j, j+1] = 1
    up = np.zeros((N, N), np.float64)
    up[np.arange(1, N), np.arange(N - 1)] = 1.0
    dn = np.zeros((N, N), np.float64)
    dn[np.arange(N - 1), np.arange(1, N)] = 1.0
    eye = np.eye(N)
    wts = np.concatenate([eye, -dn, dn, -99.0 * up, 99.0 * up],
                         axis=1).astype(np.float16)

    # GPSIMD broadcast tables for to2 = D (.) O~:  m1 = Dr*O~ ; m2 uses the
    # page-swapped view with per-page sign (-Di, +Di)
    Dr = D.real.T.astype(np.float32)  # [128 partitions, 128 stages]
    Di = D.imag.T.astype(np.float32)
    drr_t = np.empty((N, 2 * NST), np.float32)
    dsg_t = np.empty((N, 2 * NST), np.float32)
    drr_t[:, 0::2] = Dr
    drr_t[:, 1::2] = Dr
    dsg_t[:, 0::2] = -Di
    dsg_t[:, 1::2] = Di
    return coefs, E0, O0, wts, drr_t, dsg_t


def _pack(c):  # complex [128, cols] -> f32 [128, 2*cols]
    return np.concatenate([c.real, c.imag], axis=1).astype(np.float32)


def _make_in_maps(theta_in, theta_even, theta_out):
    coefs, E0, O0, wts, drr_t, dsg_t = _host_inputs(theta_in, theta_even, theta_out)
    in_maps = []
    for r in range(NCORES):
        m = {"wts": wts, "drr": drr_t, "dsg": dsg_t}
        for c in range(2):
            cols = slice(r * COLS + c * H, r * COLS + (c + 1) * H)
            m[f"xe{c}"] = _pack(E0[:, cols])
            m[f"xo{c}"] = _pack(O0[:, cols])
        for i in range(8):
            m[f"coef{i}"] = coefs[i]
        in_maps.append(m)
    return in_maps


def kernel(theta_in, theta_even, theta_out):
    from concourse.bass_utils import run_bass_kernel_spmd

    in_maps = _make_in_maps(theta_in, theta_even, theta_out)
    nc = _build_program()
    res = run_bass_kernel_spmd(nc, in_maps, list(range(NCORES)))
    out = np.zeros((N, N), np.complex64)
    for r in range(NCORES):
        for c in range(2):
            o = res.results[r][f"out{c}"]
            cols = slice(r * COLS + c * H, r * COLS + (c + 1) * H)
            out[:, cols] = o[:, :H] + 1j * o[:, H:]
    return out


# revision 12
# speedup vs baseline: 1.0823x; 1.0823x over previous
"""Photonic-mesh (NEUROPULS) chain kernel for Trainium2, 8 NeuronCores.

Sequential chain of 128 fused stages; each stage applies a per-pair 2x2
complex mix (heaters+MMIs) then a crossing that couples adjacent pairs.
Columns of the accumulated arch matrix propagate independently: 16
columns per core, split into two 8-column chains that interleave so
engines overlap.

Per stage per chain, with gauge state (E~ = E, O~ = wt*O, wt^2 = 99):
  DVE (4 custom ops, fp16 out):  te1=A(.)E~ te2=B(.)O~ to1=G(.)E~ to2=D(.)O~
  PE  (fp16, PSUM acc, exact weights I/+-S_dn/+-99*S_up):
      psE = te1+te2 + i*S_dn(to1+to2)   [i via +- half-column matmuls]
      psO = to1+to2 + i*99*S_up(te1+te2) * (1/wt gauge)
  ACT: copy psE->E~', psO->O~'  (fp32 SBUF)
All per-stage physics constants live in the per-partition fp32 DVE
scalars (exact); PE weights are exactly representable in fp16.
"""

import math

import numpy as np

import concourse.bass as bass
import concourse.mybir as mybir
from concourse.ap import AP

N = 128
NCORES = 8
COLS = 16            # columns per core
H = 8                # columns per chain (2 chains per core)
NST = 128            # C-stages (0..127); crossings after stages 0..126
NSTAGES = 129        # + projection stage

IL_MMI = 0.05
IMB = 0.005
IL_X = 0.02
CT = 0.01
WT = math.sqrt((1.0 - CT) / CT)   # wt^2 = 99 exactly

F32 = mybir.dt.float32
F16 = mybir.dt.float16

# ----------------------------------------------------------------------------
# custom DVE op: out[p,s,k] = in1[p, s*H+k]*s0[p] + in0[p,s,k]*s1[p]*(2s-1)
# with in1 = natural [re|im] view and in0 = page-swapped view this is a full
# per-partition complex scale (s0 + i*s1) (.) x in one op.
# ----------------------------------------------------------------------------
_CMULA = None


def _get_cmula():
    global _CMULA
    if _CMULA is not None:
        return _CMULA
    import concourse.dve_ops as dom
    from concourse.dve_ops import OPS, DveOp
    from concourse.dve_spec import Spec, Src0, Src1, C0, C1, SubIdx, One, lower
    from concourse.dve_uop import DveOpSpec

    name = "CMULA_NP_ANT"
    for op in OPS:  # idempotent across re-imports
        if op.name == name:
            _CMULA = op
            return op

    def _ref(in0, in1, s0, s1, imm2):
        pg = (np.arange(in0.shape[1], dtype=np.float32) * 2.0 - 1.0).reshape(1, -1, 1)
        a = np.asarray(s0, np.float32).reshape(-1, 1, 1) if np.ndim(s0) else np.float32(s0)
        b = np.asarray(s1, np.float32).reshape(-1, 1, 1) if np.ndim(s1) else np.float32(s1)
        return (np.asarray(in1, np.float32).reshape(in0.shape) * a
                + np.asarray(in0, np.float32) * b * pg).astype(np.float32)

    op = DveOp(
        name,
        Spec(body=Src1 * C0 + Src0 * C1 * (SubIdx + SubIdx - One), reference=_ref),
        subdim=True,
        uops_sha={},
    )
    OPS.append(op)
    dom._SUB_OPCODE_FOR_NAME[name] = dom._CUSTOM_DVE_ROW_BASE + len(OPS) - 1
    dom.CUSTOM_DVE_SPECS[name] = op.spec
    for ver in ("v3", "v4"):
        spec_c = DveOpSpec(name=name, opcode=dom.get_dve_sub_opcode(name),
                           uops=lower(op.spec, ver=ver), rd1_en=True)
        op.uops_sha[ver] = spec_c.sha(ver)
    _CMULA = op
    return op


def _nat3(t, h=H):
    """[P, 2, h] natural-page view of a [P, 2h] AP."""
    return AP(t.tensor, t.offset, [list(t.ap[0]), [h, 2], [1, h]])


def _swp3(t, h=H):
    """[P, 2, h] page-swapped view of a [P, 2h] AP (page0 = imag half)."""
    return AP(t.tensor, t.offset + h, [list(t.ap[0]), [-h, 2], [1, h]])


# ----------------------------------------------------------------------------
# device program (input-independent; built once)
# ----------------------------------------------------------------------------
_PROG = None


def _build_program():
    global _PROG
    if _PROG is not None:
        return _PROG
    CMULA = _get_cmula()

    import concourse.bacc as bacc
    nc = bacc.Bacc(None, target_bir_lowering=False)
    d_xe = [nc.declare_dram_parameter(f"xe{c}", [N, 2 * H], F32, isOutput=False)
            for c in range(2)]
    d_xo = [nc.declare_dram_parameter(f"xo{c}", [N, 2 * H], F32, isOutput=False)
            for c in range(2)]
    d_coef = [nc.declare_dram_parameter(f"coef{i}", [N, NSTAGES], F32, isOutput=False)
              for i in range(8)]
    d_drr = nc.declare_dram_parameter("drr", [N, 2 * NST], F32, isOutput=False)
    d_dsg = nc.declare_dram_parameter("dsg", [N, 2 * NST], F32, isOutput=False)
    d_w = nc.declare_dram_parameter("wts", [N, 5 * N], F16, isOutput=False)
    d_out = [nc.declare_dram_parameter(f"out{c}", [N, 2 * H], F32, isOutput=True)
             for c in range(2)]

    from concourse import tile

    with tile.TileContext(nc) as tc:
        with (tc.tile_pool(name="const", bufs=1) as cpool,
              tc.tile_pool(name="state", bufs=2) as spool,
              tc.tile_pool(name="tmp", bufs=3) as tpool,
              tc.tile_pool(name="ps", bufs=2, space="PSUM") as ppool):
            coefT = cpool.tile([N, 8 * NSTAGES], F32, tag="coef")
            drr = cpool.tile([N, 2 * NST], F32, tag="drr")
            dsg = cpool.tile([N, 2 * NST], F32, tag="dsg")
            wts = cpool.tile([N, 5 * N], F16, tag="wts")
            for i in range(8):
                nc.sync.dma_start(coefT[:, i * NSTAGES:(i + 1) * NSTAGES],
                                  d_coef[i][:])
            nc.sync.dma_start(drr[:], d_drr[:])
            nc.sync.dma_start(dsg[:], d_dsg[:])
            nc.sync.dma_start(wts[:], d_w[:])
            coef = [coefT[:, i * NSTAGES:(i + 1) * NSTAGES] for i in range(8)]
            wI = wts[:, 0 * N:1 * N]
            wDnM = wts[:, 1 * N:2 * N]   # -S_dn
            wDnP = wts[:, 2 * N:3 * N]   # +S_dn
            wUpM = wts[:, 3 * N:4 * N]   # -99*S_up
            wUpP = wts[:, 4 * N:5 * N]   # +99*S_up

            xeo, outT = [], []
            for c in range(2):
                t1 = spool.tile([N, 4 * H], F32, name=f"xeo_{c}", tag=f"xeo{c}")
                t3 = cpool.tile([N, 2 * H], F32, name=f"outT_{c}", tag=f"outT{c}")
                nc.sync.dma_start(t1[:, 0:2 * H], d_xe[c][:])
                nc.sync.dma_start(t1[:, 2 * H:4 * H], d_xo[c][:])
                xeo.append(t1)
                outT.append(t3)

            def cmul(dst, src, cr, ci):
                return nc.vector._custom_dve(CMULA, out=_nat3(dst[:]),
                                             in0=_swp3(src), in1=src,
                                             s0=cr, s1=ci)

            def bview(t, k):
                b = t[:]
                return AP(b.tensor, b.offset + 2 * k, [list(b.ap[0]), [1, 2], [0, H]])

            for k in range(NST):  # C-stages 0..127
                cc = [coef[i][:, k:k + 1] for i in range(8)]
                last = k == NST - 1
                for c in range(2):
                    xe_v = xeo[c][:, 0:2 * H]
                    xo_v = xeo[c][:, 2 * H:4 * H]
                    te1 = tpool.tile([N, 2 * H], F16, name=f"te1_{c}", tag=f"te1{c}")
                    te2 = tpool.tile([N, 2 * H], F16, name=f"te2_{c}", tag=f"te2{c}")
                    to1 = tpool.tile([N, 2 * H], F16, name=f"to1_{c}", tag=f"to1{c}")
                    to2 = tpool.tile([N, 2 * H], F16, name=f"to2_{c}", tag=f"to2{c}")
                    cmul(te1, xe_v, cc[0], cc[1])   # A (.) E~
                    cmul(te2, xo_v, cc[2], cc[3])   # B (.) O~
                    cmul(to1, xe_v, cc[4], cc[5])   # G (.) E~
                    cmul(to2, xo_v, cc[6], cc[7])   # D (.) O~
                    ps = ppool.tile([N, 4 * H], F32, name=f"ps_{c}", tag=f"ps{c}")
                    psE = ps[:, 0:2 * H]
                    psO = ps[:, 2 * H:4 * H]
                    # psE = te1+te2 (+ crossing i*S_dn(to1+to2))
                    nc.tensor.matmul(psE, wI, te1[:], start=True, stop=False)
                    nc.tensor.matmul(psE, wI, te2[:], start=False, stop=last)
                    if not last:
                        for t in (to1, to2):
                            nc.tensor.matmul(ps[:, 0:H], wDnM, t[:, H:2 * H],
                                             start=False, stop=False)
                        for i, t in enumerate((to1, to2)):
                            nc.tensor.matmul(ps[:, H:2 * H], wDnP, t[:, 0:H],
                                             start=False, stop=(i == 1))
                    # psO = to1+to2 (+ crossing i*99*S_up(te1+te2))
                    nc.tensor.matmul(psO, wI, to1[:], start=True, stop=False)
                    nc.tensor.matmul(psO, wI, to2[:], start=False, stop=last)
                    if not last:
                        for t in (te1, te2):
                            nc.tensor.matmul(ps[:, 2 * H:3 * H], wUpM, t[:, H:2 * H],
                                             start=False, stop=False)
                        for i, t in enumerate((te1, te2)):
                            nc.tensor.matmul(ps[:, 3 * H:4 * H], wUpP, t[:, 0:H],
                                             start=False, stop=(i == 1))
                    xeo_n = spool.tile([N, 4 * H], F32, name=f"xeon_{c}", tag=f"xeo{c}")
                    nc.scalar.copy(xeo_n[:], ps[:])
                    xeo[c] = xeo_n

            # projection stage
            cc = [coef[i][:, NST:NST + 1] for i in range(4)]
            for c in range(2):
                tp1 = tpool.tile([N, 2 * H], F16, name=f"tp1_{c}", tag=f"te1{c}")
                tp2 = tpool.tile([N, 2 * H], F16, name=f"tp2_{c}", tag=f"te2{c}")
                cmul(tp1, xeo[c][:, 0:2 * H], cc[0], cc[1])
                cmul(tp2, xeo[c][:, 2 * H:4 * H], cc[2], cc[3])
                psP = ppool.tile([N, 2 * H], F32, name=f"psP_{c}", tag=f"psP{c}")
                nc.tensor.matmul(psP[:], wI, tp1[:], start=True, stop=False)
                nc.tensor.matmul(psP[:], wI, tp2[:], start=False, stop=True)
                nc.scalar.copy(outT[c][:], psP[:])
                nc.sync.dma_start(d_out[c][:], outT[c][:])

    nc.finalize()
    _PROG = nc
    return _PROG


# ----------------------------------------------------------------------------
# host-side coefficient construction
# ----------------------------------------------------------------------------
def _host_inputs(theta_in, theta_even, theta_out):
    theta_in = np.asarray(theta_in, np.float64)
    theta_even = np.asarray(theta_even, np.float64)
    theta_out = np.asarray(theta_out, np.float64)

    aM = math.sqrt(1.0 - IL_MMI)
    bp = aM * math.sqrt(0.5 + IMB)
    bq = aM * math.sqrt(0.5 - IMB)
    B = np.array([[bp, 1j * bq], [1j * bq, bp]], np.complex128)
    aX = math.sqrt(1.0 - IL_X)
    u = aX * math.sqrt(CT)
    vv = aX * math.sqrt(1.0 - CT)

    ph = np.exp(1j * theta_even)  # [255, 128]

    Cs = np.zeros((NSTAGES, N, 2, 2), np.complex128)
    # stage 0: B @ diag(a0, 1)
    Cs[0, :, :, 0] = B[:, 0][None, :] * ph[0][:, None]
    Cs[0, :, :, 1] = B[:, 1][None, :]
    # stages 1..126: (B @ diag(b,1)) @ (B @ diag(a,1)),  a=ph[2i-1], b=ph[2i]
    i = np.arange(1, N - 1)
    a = ph[2 * i - 1]  # [126, 128]
    b = ph[2 * i]
    T1 = np.zeros((N - 2, N, 2, 2), np.complex128)
    T1[:, :, :, 0] = B[:, 0][None, None, :] * a[:, :, None]
    T1[:, :, :, 1] = B[:, 1][None, None, :]
    T2 = np.zeros_like(T1)
    T2[:, :, :, 0] = B[:, 0][None, None, :] * b[:, :, None]
    T2[:, :, :, 1] = B[:, 1][None, None, :]
    Cs[1:N - 1] = np.einsum("sjab,sjbc->sjac", T2, T1)
    # stage 127: half epilogue B @ diag(ph[253], 1)
    Cs[N - 1, :, :, 0] = B[:, 0][None, :] * ph[2 * N - 3][:, None]
    Cs[N - 1, :, :, 1] = B[:, 1][None, :]
    # stage 128: projection  out = f0*E + f1*O
    f0 = np.exp(1j * theta_out) * bp * ph[2 * N - 2]
    f1 = np.exp(1j * theta_out) * (1j * bq)
    Cs[N, :, 0, 0] = f0
    Cs[N, :, 0, 1] = f1

    # fold crossing scalars/corners of K-stage s (s=0..126) into stage s+1
    dE = np.full(N, u); dE[0] = vv
    dO = np.full(N, u); dO[N - 1] = vv
    Cs[1:N, :, :, 0] *= dE[None, :, None]
    Cs[1:N, :, :, 1] *= dO[None, :, None]

    # gauge + normalization:
    #   state O~ = wt*O;  per-stage amplitude renorm g_k keeps fp16 in range.
    # device coefficients per stage k:
    #   A = g*C00, B = g*C01/wt, G = g*wt*C10, D = g*C11
    # projection: A = C00/Pg, B = C01/(wt*Pg)   (Pg = prod of g_k)
    A = np.zeros((NST + 1, N), np.complex128)
    Bc = np.zeros((NST + 1, N), np.complex128)
    G = np.zeros((NST, N), np.complex128)
    D = np.zeros((NST, N), np.complex128)
    Pg = 1.0
    for k in range(NST):
        g = 1.0 / (aM * aM * aX)  # typical per-stage amplitude loss
        A[k] = g * Cs[k, :, 0, 0]
        Bc[k] = g * Cs[k, :, 0, 1] / WT
        G[k] = g * WT * Cs[k, :, 1, 0]
        D[k] = g * Cs[k, :, 1, 1]
        Pg *= g
    A[NST] = Cs[NST, :, 0, 0] / Pg
    Bc[NST] = Cs[NST, :, 0, 1] / (WT * Pg)

    coefs = [np.ascontiguousarray(x.astype(np.float32)) for x in (
        A.T.real, A.T.imag, Bc.T.real, Bc.T.imag,
        np.concatenate([G, G[:1]]).T.real, np.concatenate([G, G[:1]]).T.imag,
        np.concatenate([D, D[:1]]).T.real, np.concatenate([D, D[:1]]).T.imag,
    )]

    # initial state: columns of MMI_IN @ diag(exp(i theta_in)); O~ = wt*O
    din = np.exp(1j * theta_in)
    E0 = np.zeros((N, N), np.complex128)
    O0 = np.zeros((N, N), np.complex128)
    E0[np.arange(N), np.arange(N)] = bp * din
    O0[np.arange(N), np.arange(N)] = WT * (1j * bq) * din

    # PE weights (all exact in fp16):
    #   psA[j] = x[j+1]  (S_up):  lhsT up[j+1, j] = 1
    #   psB[j] = x[j-1]  (S_dn):  lhsT dn[j, j+1] = 1
    up = np.zeros((N, N), np.float64)
    up[np.arange(1, N), np.arange(N - 1)] = 1.0
    dn = np.zeros((N, N), np.float64)
    dn[np.arange(N - 1), np.arange(1, N)] = 1.0
    eye = np.eye(N)
    wts = np.concatenate([eye, -dn, dn, -99.0 * up, 99.0 * up],
                         axis=1).astype(np.float16)

    # GPSIMD broadcast tables for to2 = D (.) O~:  m1 = Dr*O~ ; m2 uses the
    # page-swapped view with per-page sign (-Di, +Di)
    Dr = D.real.T.astype(np.float32)  # [128 partitions, 128 stages]
    Di = D.imag.T.astype(np.float32)
    drr_t = np.empty((N, 2 * NST), np.float32)
    dsg_t = np.empty((N, 2 * NST), np.float32)
    drr_t[:, 0::2] = Dr
    drr_t[:, 1::2] = Dr
    dsg_t[:, 0::2] = -Di
    dsg_t[:, 1::2] = Di
    return coefs, E0, O0, wts, drr_t, dsg_t


def _pack(c):  # complex [128, cols] -> f32 [128, 2*cols]
    return np.concatenate([c.real, c.imag], axis=1).astype(np.float32)


def _make_in_maps(theta_in, theta_even, theta_out):
    coefs, E0, O0, wts, drr_t, dsg_t = _host_inputs(theta_in, theta_even, theta_out)
    in_maps = []
    for r in range(NCORES):
        m = {"wts": wts, "drr": drr_t, "dsg": dsg_t}
        for c in range(2):
            cols = slice(r * COLS + c * H, r * COLS + (c + 1) * H)
            m[f"xe{c}"] = _pack(E0[:, cols])
            m[f"xo{c}"] = _pack(O0[:, cols])
        for i in range(8):
            m[f"coef{i}"] = coefs[i]
        in_maps.append(m)
    return in_maps


def kernel(theta_in, theta_even, theta_out):
    from concourse.bass_utils import run_bass_kernel_spmd

    in_maps = _make_in_maps(theta_in, theta_even, theta_out)
    nc = _build_program()
    res = run_bass_kernel_spmd(nc, in_maps, list(range(NCORES)))
    out = np.zeros((N, N), np.complex64)
    for r in range(NCORES):
        for c in range(2):
            o = res.results[r][f"out{c}"]
            cols = slice(r * COLS + c * H, r * COLS + (c + 1) * H)
            out[:, cols] = o[:, :H] + 1j * o[:, H:]
    return out
